# revision 3
# baseline (speedup 1.0000x reference)
"""Multi-head attention (B=2, S=2048, H=1024, NH=16, HD=64) on 8 TRN2 cores.

Sharding: tensor-parallel over heads — 2 heads per core. Each core:
  - gets the full (transposed) hidden_states xT [H, B*S] in bf16
  - computes qT/kT [128, 4096] and v [4096, 128] for its 2 heads
  - computes attention locally (scores transposed: [k_pos, q] layout so
    softmax denominators come from a ones-matmul, no cross-partition ops)
  - computes a partial output  out_c = ctx_c @ Wo_c^T  [4096, 1024]
Host sums the 8 partials and adds bo.

All matmuls in bf16 (fp32 accumulation in PSUM); softmax exp in fp32 on
the scalar engine.
"""

import os
import numpy as np
import ml_dtypes

import concourse.bass as bass
import concourse.tile as tile
import concourse.mybir as mybir
from concourse import bacc
from concourse import bass_utils

F32 = mybir.dt.float32
BF16 = mybir.dt.bfloat16
NPBF16 = ml_dtypes.bfloat16

B = 2
S = 2048
H = 1024
NH = 16
HD = 64
NCORES = 8
HPC = NH // NCORES          # heads per core = 2
DSH = HPC * HD              # sharded feature dim per core = 128
ST = B * S                  # total tokens = 4096

NSB = ST // 512             # 8 s-blocks of 512 tokens
NKT_S = S // 128            # 16 k-tiles per batch in attention
NQB = S // 512              # 4 q-blocks per batch


def _build(n_kt: int):
    """Build + compile the Bass module. n_kt=8 (no bias) or 9 (bias rows)."""
    nc = bacc.Bacc("TRN2", target_bir_lowering=False, debug=False,
                   enable_asserts=True, num_devices=NCORES)

    xT = nc.dram_tensor("xT", [n_kt * 128, ST], BF16, kind="ExternalInput")
    wq = nc.dram_tensor("wq", [n_kt * 128, DSH], BF16, kind="ExternalInput")
    wk = nc.dram_tensor("wk", [n_kt * 128, DSH], BF16, kind="ExternalInput")
    wv = nc.dram_tensor("wv", [n_kt * 128, DSH], BF16, kind="ExternalInput")
    wo = nc.dram_tensor("wo", [DSH, H], BF16, kind="ExternalInput")
    out = nc.dram_tensor("out", [ST, H], F32, kind="ExternalOutput")
    xT, wq, wk, wv, wo, out = (t.ap() for t in (xT, wq, wk, wv, wo, out))

    with tile.TileContext(nc) as tc:
        _emit(tc, n_kt, xT, wq, wk, wv, wo, out)
    nc.compile()
    return nc


def _emit(tc, n_kt, xT, wq, wk, wv, wo, out):
    nc = tc.nc
    ctx_pools = []

    def pool(name, bufs, space="SBUF"):
        p = tc.alloc_tile_pool(name=name, bufs=bufs, space=space)
        ctx_pools.append(p)
        return p

    # --- pools -----------------------------------------------------------
    xp = pool("x", n_kt * NSB)                 # x tiles [128, 512] bf16
    pw = pool("w", 3 * n_kt)                   # weight tiles [128, 128] bf16
    pwo = pool("wo", 1)                        # [128, 1024] bf16
    pqk = pool("qk", 2)                        # qT/kT [128, 4096] bf16
    pv = pool("v", 1)                          # v [128, 32, 128] bf16
    pctxsb = pool("ctxT", 1)                   # ctxT [128, 4096] bf16
    pones = pool("ones", 1)                    # [128, 64] bf16
    pexp = pool("exp", 6)                      # expT [128, 1024] bf16
    prec = pool("rec", 2)                      # recT [128, 512] f32
    pout = pool("outsb", 4)                    # out staging [128, 1024] f32
    # PSUM: big pool 3x2 banks + attention pool 2x1 bank = 8 banks
    PP = pool("pp", 3, space="PSUM")           # [128, 1024] f32
    PA = pool("pa", 2, space="PSUM")           # [128, 512] f32

    # --- load weights and x ---------------------------------------------
    wo_t = pwo.tile([128, H], BF16)
    nc.sync.dma_start(wo_t[:], wo[:, :])
    wq_t, wk_t, wv_t = [], [], []
    for kt in range(n_kt):
        for lst, src in ((wq_t, wq), (wk_t, wk), (wv_t, wv)):
            t = pw.tile([128, DSH], BF16)
            nc.sync.dma_start(t[:], src[kt * 128:(kt + 1) * 128, :])
            lst.append(t)

    x_t = [[None] * NSB for _ in range(n_kt)]
    for sb in range(NSB):
        for kt in range(n_kt):
            t = xp.tile([128, 512], BF16)
            nc.sync.dma_start(t[:], xT[kt * 128:(kt + 1) * 128,
                                       sb * 512:(sb + 1) * 512])
            x_t[kt][sb] = t

    ones_t = pones.tile([128, 64], BF16)
    nc.vector.memset(ones_t[:], 1.0)

    qT = pqk.tile([128, ST], BF16, tag="qk")
    kT = pqk.tile([128, ST], BF16, tag="qk")
    v_sb = pv.tile([128, NSB * 4, DSH], BF16)
    ctxT = pctxsb.tile([128, ST], BF16)

    # --- projections -----------------------------------------------------
    # qT/kT: out[d(128), s] ; lhsT = w[kt] [h,d], rhs = x[kt][sb] [h,s]
    for sbp in range(NSB // 2):                # pairs of s-blocks share a tile
        for w_list, dst in ((wq_t, qT), (wk_t, kT)):
            ps = PP.tile([128, 1024], F32, tag="pp")
            for half in range(2):
                sb = sbp * 2 + half
                for kt in range(n_kt):
                    nc.tensor.matmul(ps[:, half * 512:(half + 1) * 512],
                                     w_list[kt][:], x_t[kt][sb][:],
                                     start=(kt == 0), stop=(kt == n_kt - 1))
            nc.any.tensor_copy(dst[:, sbp * 1024:(sbp + 1) * 1024], ps[:])
        # v: out[s(128), d(128)] ; lhsT = x[kt][sb] slice [h, s128], rhs = wv[kt]
        ps = PP.tile([128, 1024], F32, tag="pp")
        for half in range(2):
            sb = sbp * 2 + half
            for ssb in range(4):
                o = half * 512 + ssb * 128
                for kt in range(n_kt):
                    nc.tensor.matmul(ps[:, o:o + 128],
                                     x_t[kt][sb][:, ssb * 128:(ssb + 1) * 128],
                                     wv_t[kt][:],
                                     start=(kt == 0), stop=(kt == n_kt - 1))
        nc.any.tensor_copy(v_sb[:, sbp * 8:(sbp + 1) * 8, :], ps[:])

    # --- attention + out-projection, per (batch, q-block) ---------------
    for b in range(B):
        for qb in range(NQB):
            q0 = b * S + qb * 512              # global column of this q-block
            ctx_ps = PA.tile([128, 512], F32, tag="pa")
            den_ps = PA.tile([128, 512], F32, tag="pa")
            for ktp in range(NKT_S // 2):
                expt = []
                for h in range(2):             # scores, row-packed pairs
                    sc = PP.tile([128, 1024], F32, tag="pp")
                    for j in range(2):
                        kt = ktp * 2 + j
                        k0 = b * S + kt * 128
                        nc.tensor.matmul(
                            sc[:, j * 512:(j + 1) * 512],
                            kT[h * 64:(h + 1) * 64, k0:k0 + 128],
                            qT[h * 64:(h + 1) * 64, q0:q0 + 512],
                            start=True, stop=True)
                    e = pexp.tile([128, 1024], BF16)
                    nc.scalar.activation(e[:], sc[:],
                                         mybir.ActivationFunctionType.Exp,
                                         scale=0.125)
                    expt.append(e)
                for j in range(2):             # ctx + denom accumulate
                    kt = ktp * 2 + j
                    g = b * NKT_S + kt
                    st = (ktp == 0 and j == 0)
                    sp = (ktp == NKT_S // 2 - 1 and j == 1)
                    for h in range(2):
                        nc.tensor.matmul(ctx_ps[h * 64:(h + 1) * 64, :],
                                         v_sb[:, g, h * 64:(h + 1) * 64],
                                         expt[h][:, j * 512:(j + 1) * 512],
                                         start=st, stop=sp)
                    for h in range(2):
                        nc.tensor.matmul(den_ps[h * 64:(h + 1) * 64, :],
                                         ones_t[:],
                                         expt[h][:, j * 512:(j + 1) * 512],
                                         start=st, stop=sp)
            rec = prec.tile([128, 512], F32)
            nc.vector.reciprocal(rec[:], den_ps[:])
            nc.vector.tensor_mul(ctxT[:, q0:q0 + 512], ctx_ps[:], rec[:])

            # out rows q0..q0+512 : lhsT = ctxT col-slice, rhs = wo
            for ssb in range(4):
                c0 = q0 + ssb * 128
                ps = PP.tile([128, 1024], F32, tag="pp")
                for e in range(2):
                    nc.tensor.matmul(ps[:, e * 512:(e + 1) * 512],
                                     ctxT[:, c0:c0 + 128],
                                     wo_t[:, e * 512:(e + 1) * 512],
                                     start=True, stop=True)
                ot = pout.tile([128, H], F32)
                nc.any.tensor_copy(ot[:], ps[:])
                nc.sync.dma_start(out[c0:c0 + 128, :], ot[:])

    for p in reversed(ctx_pools):
        p.release()


_CACHE = {}


def _get_nc(n_kt):
    if n_kt not in _CACHE:
        _CACHE[n_kt] = _build(n_kt)
    return _CACHE[n_kt]


def _prep_inputs(hidden_states, Wq, bq, Wk, bk, Wv, bv, Wo, bo):
    x = np.ascontiguousarray(np.asarray(hidden_states, np.float32)
                             .reshape(ST, H))
    bias = not (np.all(bq == 0) and np.all(bk == 0) and np.all(bv == 0))
    n_kt = 9 if bias else 8
    xTn = np.zeros((n_kt * 128, ST), np.float32)
    xTn[:H] = x.T
    if bias:
        xTn[H] = 1.0
    xTn = xTn.astype(NPBF16)

    in_maps = []
    for c in range(NCORES):
        rows = slice(c * DSH, (c + 1) * DSH)
        m = {"xT": xTn}
        for name, W, bvec in (("wq", Wq, bq), ("wk", Wk, bk), ("wv", Wv, bv)):
            wt = np.zeros((n_kt * 128, DSH), np.float32)
            wt[:H] = np.asarray(W, np.float32)[rows, :].T
            if bias:
                wt[H] = np.asarray(bvec, np.float32)[rows]
            m[name] = wt.astype(NPBF16)
        m["wo"] = np.ascontiguousarray(
            np.asarray(Wo, np.float32)[:, rows].T).astype(NPBF16)
        in_maps.append(m)
    return n_kt, in_maps


def kernel(hidden_states, Wq, bq, Wk, bk, Wv, bv, Wo, bo, _return_extras=False):
    n_kt, in_maps = _prep_inputs(hidden_states, Wq, bq, Wk, bk, Wv, bv, Wo, bo)
    nc = _get_nc(n_kt)
    res = bass_utils.run_bass_kernel_spmd(nc, in_maps,
                                          core_ids=list(range(NCORES)))
    acc = res.results[0]["out"].astype(np.float64)
    for c in range(1, NCORES):
        acc += res.results[c]["out"]
    acc += np.asarray(bo, np.float64)
    outv = acc.astype(np.float32).reshape(B, S, H)
    if _return_extras:
        return outv, (nc, in_maps, res)
    return outv


# revision 4
# speedup vs baseline: 6.6081x; 6.6081x over previous
"""Multi-head attention (B=2, S=2048, H=1024, NH=16, HD=64) on 8 TRN2 cores.

Sharding: tensor-parallel over heads — 2 heads per core. Each core:
  - gets the full (transposed) hidden_states xT [H, B*S] in bf16
  - computes qT/kT [128, 4096] and v [4096, 128] for its 2 heads
  - computes attention locally (scores transposed: [k_pos, q] layout so
    softmax denominators come from a ones-matmul, no cross-partition ops)
  - computes a partial output  out_c = ctx_c @ Wo_c^T  [4096, 1024]
Host sums the 8 partials and adds bo.

All matmuls in bf16 (fp32 accumulation in PSUM); softmax exp in fp32 on
the scalar engine.
"""

import os
import numpy as np
import ml_dtypes

import concourse.bass as bass
import concourse.tile as tile
import concourse.mybir as mybir
from concourse import bacc
from concourse import bass_utils

F32 = mybir.dt.float32
BF16 = mybir.dt.bfloat16
NPBF16 = ml_dtypes.bfloat16

B = 2
S = 2048
H = 1024
NH = 16
HD = 64
NCORES = 8
HPC = NH // NCORES          # heads per core = 2
DSH = HPC * HD              # sharded feature dim per core = 128
ST = B * S                  # total tokens = 4096

NSB = ST // 512             # 8 s-blocks of 512 tokens
NKT_S = S // 128            # 16 k-tiles per batch in attention
NQB = S // 512              # 4 q-blocks per batch


def _build(n_kt: int, reps: int = 1):
    """Build + compile the Bass module. n_kt=8 (no bias) or 9 (bias rows).
    reps>1 repeats the whole kernel body (for device-time measurement)."""
    nc = bacc.Bacc("TRN2", target_bir_lowering=False, debug=False,
                   enable_asserts=True, num_devices=NCORES)

    xT = nc.dram_tensor("xT", [n_kt * 128, ST], BF16, kind="ExternalInput")
    wq = nc.dram_tensor("wq", [n_kt * 128, DSH], BF16, kind="ExternalInput")
    wk = nc.dram_tensor("wk", [n_kt * 128, DSH], BF16, kind="ExternalInput")
    wv = nc.dram_tensor("wv", [n_kt * 128, DSH], BF16, kind="ExternalInput")
    wo = nc.dram_tensor("wo", [DSH, H], BF16, kind="ExternalInput")
    out = nc.dram_tensor("out", [ST, H], F32, kind="ExternalOutput")
    xT, wq, wk, wv, wo, out = (t.ap() for t in (xT, wq, wk, wv, wo, out))

    with tile.TileContext(nc) as tc:
        for _ in range(reps):
            _emit(tc, n_kt, xT, wq, wk, wv, wo, out)
    nc.compile()
    return nc


def _emit(tc, n_kt, xT, wq, wk, wv, wo, out):
    nc = tc.nc
    ctx_pools = []

    def pool(name, bufs, space="SBUF"):
        p = tc.alloc_tile_pool(name=name, bufs=bufs, space=space)
        ctx_pools.append(p)
        return p

    # --- pools -----------------------------------------------------------
    xp = pool("x", n_kt * NSB)                 # x tiles [128, 512] bf16
    pw = pool("w", 3 * n_kt)                   # weight tiles [128, 128] bf16
    pwo = pool("wo", 1)                        # [128, 1024] bf16
    pqk = pool("qk", 2)                        # qT/kT [128, 4096] bf16
    pv = pool("v", 1)                          # v [128, 32, 128] bf16
    pctxsb = pool("ctxT", 1)                   # ctxT [128, 4096] bf16
    pones = pool("ones", 1)                    # [128, 64] bf16
    pexp = pool("exp", 6)                      # expT [128, 1024] bf16
    prec = pool("rec", 2)                      # recT [128, 512] f32
    pout = pool("outsb", 4)                    # out staging [128, 1024] f32
    # PSUM: big pool 3x2 banks + attention pool 2x1 bank = 8 banks
    PP = pool("pp", 3, space="PSUM")           # [128, 1024] f32
    PA = pool("pa", 2, space="PSUM")           # [128, 512] f32

    # --- load weights and x ---------------------------------------------
    wo_t = pwo.tile([128, H], BF16)
    nc.sync.dma_start(wo_t[:], wo[:, :])
    wq_t, wk_t, wv_t = [], [], []
    for kt in range(n_kt):
        for lst, src in ((wq_t, wq), (wk_t, wk), (wv_t, wv)):
            t = pw.tile([128, DSH], BF16)
            nc.sync.dma_start(t[:], src[kt * 128:(kt + 1) * 128, :])
            lst.append(t)

    x_t = [[None] * NSB for _ in range(n_kt)]
    for sb in range(NSB):
        for kt in range(n_kt):
            t = xp.tile([128, 512], BF16)
            nc.sync.dma_start(t[:], xT[kt * 128:(kt + 1) * 128,
                                       sb * 512:(sb + 1) * 512])
            x_t[kt][sb] = t

    ones_t = pones.tile([128, 64], BF16)
    nc.vector.memset(ones_t[:], 1.0)

    qT = pqk.tile([128, ST], BF16, tag="qk")
    kT = pqk.tile([128, ST], BF16, tag="qk")
    v_sb = pv.tile([128, NSB * 4, DSH], BF16)
    ctxT = pctxsb.tile([128, ST], BF16)

    # --- projections -----------------------------------------------------
    # qT/kT: out[d(128), s] ; lhsT = w[kt] [h,d], rhs = x[kt][sb] [h,s]
    for sbp in range(NSB // 2):                # pairs of s-blocks share a tile
        for w_list, dst in ((wq_t, qT), (wk_t, kT)):
            ps = PP.tile([128, 1024], F32, tag="pp")
            for half in range(2):
                sb = sbp * 2 + half
                for kt in range(n_kt):
                    nc.tensor.matmul(ps[:, half * 512:(half + 1) * 512],
                                     w_list[kt][:], x_t[kt][sb][:],
                                     start=(kt == 0), stop=(kt == n_kt - 1))
            nc.any.tensor_copy(dst[:, sbp * 1024:(sbp + 1) * 1024], ps[:])
        # v: out[s(128), d(128)] ; lhsT = x[kt][sb] slice [h, s128], rhs = wv[kt]
        ps = PP.tile([128, 1024], F32, tag="pp")
        for half in range(2):
            sb = sbp * 2 + half
            for ssb in range(4):
                o = half * 512 + ssb * 128
                for kt in range(n_kt):
                    nc.tensor.matmul(ps[:, o:o + 128],
                                     x_t[kt][sb][:, ssb * 128:(ssb + 1) * 128],
                                     wv_t[kt][:],
                                     start=(kt == 0), stop=(kt == n_kt - 1))
        nc.any.tensor_copy(v_sb[:, sbp * 8:(sbp + 1) * 8, :], ps[:])

    # --- attention + out-projection, per (batch, q-block) ---------------
    for b in range(B):
        for qb in range(NQB):
            q0 = b * S + qb * 512              # global column of this q-block
            ctx_ps = PA.tile([128, 512], F32, tag="pa")
            den_ps = PA.tile([128, 512], F32, tag="pa")
            for ktp in range(NKT_S // 2):
                expt = []
                for h in range(2):             # scores, row-packed pairs
                    sc = PP.tile([128, 1024], F32, tag="pp")
                    for j in range(2):
                        kt = ktp * 2 + j
                        k0 = b * S + kt * 128
                        nc.tensor.matmul(
                            sc[:, j * 512:(j + 1) * 512],
                            kT[h * 64:(h + 1) * 64, k0:k0 + 128],
                            qT[h * 64:(h + 1) * 64, q0:q0 + 512],
                            start=True, stop=True)
                    e = pexp.tile([128, 1024], BF16)
                    nc.scalar.activation(e[:], sc[:],
                                         mybir.ActivationFunctionType.Exp,
                                         scale=0.125)
                    expt.append(e)
                for j in range(2):             # ctx + denom accumulate
                    kt = ktp * 2 + j
                    g = b * NKT_S + kt
                    st = (ktp == 0 and j == 0)
                    sp = (ktp == NKT_S // 2 - 1 and j == 1)
                    for h in range(2):
                        nc.tensor.matmul(ctx_ps[h * 64:(h + 1) * 64, :],
                                         v_sb[:, g, h * 64:(h + 1) * 64],
                                         expt[h][:, j * 512:(j + 1) * 512],
                                         start=st, stop=sp)
                    for h in range(2):
                        nc.tensor.matmul(den_ps[h * 64:(h + 1) * 64, :],
                                         ones_t[:],
                                         expt[h][:, j * 512:(j + 1) * 512],
                                         start=st, stop=sp)
            rec = prec.tile([128, 512], F32)
            nc.vector.reciprocal(rec[:], den_ps[:])
            nc.vector.tensor_mul(ctxT[:, q0:q0 + 512], ctx_ps[:], rec[:])

            # out rows q0..q0+512 : lhsT = ctxT col-slice, rhs = wo
            for ssb in range(4):
                c0 = q0 + ssb * 128
                ps = PP.tile([128, 1024], F32, tag="pp")
                for e in range(2):
                    nc.tensor.matmul(ps[:, e * 512:(e + 1) * 512],
                                     ctxT[:, c0:c0 + 128],
                                     wo_t[:, e * 512:(e + 1) * 512],
                                     start=True, stop=True)
                ot = pout.tile([128, H], F32)
                nc.any.tensor_copy(ot[:], ps[:])
                nc.sync.dma_start(out[c0:c0 + 128, :], ot[:])

    for p in reversed(ctx_pools):
        p.release()


_CACHE = {}


def _get_nc(n_kt):
    if n_kt not in _CACHE:
        _CACHE[n_kt] = _build(n_kt)
    return _CACHE[n_kt]


def _prep_inputs(hidden_states, Wq, bq, Wk, bk, Wv, bv, Wo, bo):
    x = np.ascontiguousarray(np.asarray(hidden_states, np.float32)
                             .reshape(ST, H))
    bias = not (np.all(bq == 0) and np.all(bk == 0) and np.all(bv == 0))
    n_kt = 9 if bias else 8
    xTn = np.zeros((n_kt * 128, ST), np.float32)
    xTn[:H] = x.T
    if bias:
        xTn[H] = 1.0
    xTn = xTn.astype(NPBF16)

    in_maps = []
    for c in range(NCORES):
        rows = slice(c * DSH, (c + 1) * DSH)
        m = {"xT": xTn}
        for name, W, bvec in (("wq", Wq, bq), ("wk", Wk, bk), ("wv", Wv, bv)):
            wt = np.zeros((n_kt * 128, DSH), np.float32)
            wt[:H] = np.asarray(W, np.float32)[rows, :].T
            if bias:
                wt[H] = np.asarray(bvec, np.float32)[rows]
            m[name] = wt.astype(NPBF16)
        m["wo"] = np.ascontiguousarray(
            np.asarray(Wo, np.float32)[:, rows].T).astype(NPBF16)
        in_maps.append(m)
    return n_kt, in_maps


def kernel(hidden_states, Wq, bq, Wk, bk, Wv, bv, Wo, bo, _return_extras=False):
    n_kt, in_maps = _prep_inputs(hidden_states, Wq, bq, Wk, bk, Wv, bv, Wo, bo)
    nc = _get_nc(n_kt)
    res = bass_utils.run_bass_kernel_spmd(nc, in_maps,
                                          core_ids=list(range(NCORES)))
    acc = res.results[0]["out"].astype(np.float64)
    for c in range(1, NCORES):
        acc += res.results[c]["out"]
    acc += np.asarray(bo, np.float64)
    outv = acc.astype(np.float32).reshape(B, S, H)
    if _return_extras:
        return outv, (nc, in_maps, res)
    return outv


# revision 5
# speedup vs baseline: 105.6597x; 15.9894x over previous
"""Multi-head attention (B=2, S=2048, H=1024, NH=16, HD=64) on 8 TRN2 cores.

Sharding: tensor-parallel over heads — 2 heads per core. Each core:
  - gets the full (transposed) hidden_states xT [H, B*S] in bf16
  - computes qT/kT [128, 4096] and v [4096, 128] for its 2 heads
  - computes attention locally (scores transposed: [k_pos, q] layout so
    softmax denominators come from a ones-matmul, no cross-partition ops)
  - computes a partial output  out_c = ctx_c @ Wo_c^T  [4096, 1024]
Host sums the 8 partials and adds bo.

All matmuls in bf16 (fp32 accumulation in PSUM); softmax exp in fp32 on
the scalar engine.
"""

import os
import numpy as np
import ml_dtypes

import concourse.bass as bass
import concourse.tile as tile
import concourse.mybir as mybir
from concourse import bacc
from concourse import bass_utils

F32 = mybir.dt.float32
BF16 = mybir.dt.bfloat16
NPBF16 = ml_dtypes.bfloat16

B = 2
S = 2048
H = 1024
NH = 16
HD = 64
NCORES = 8
HPC = NH // NCORES          # heads per core = 2
DSH = HPC * HD              # sharded feature dim per core = 128
ST = B * S                  # total tokens = 4096

NSB = ST // 512             # 8 s-blocks of 512 tokens
NKT_S = S // 128            # 16 k-tiles per batch in attention
NQB = S // 512              # 4 q-blocks per batch


def _build(n_kt: int, reps: int = 1):
    """Build + compile the Bass module. n_kt=8 (no bias) or 9 (bias rows).
    reps>1 repeats the whole kernel body (for device-time measurement)."""
    nc = bacc.Bacc("TRN2", target_bir_lowering=False, debug=False,
                   enable_asserts=True, num_devices=NCORES)

    xT = nc.dram_tensor("xT", [n_kt * 128, ST], BF16, kind="ExternalInput")
    wq = nc.dram_tensor("wq", [n_kt * 128, DSH], BF16, kind="ExternalInput")
    wk = nc.dram_tensor("wk", [n_kt * 128, DSH], BF16, kind="ExternalInput")
    wv = nc.dram_tensor("wv", [n_kt * 128, DSH], BF16, kind="ExternalInput")
    wo = nc.dram_tensor("wo", [DSH, H], BF16, kind="ExternalInput")
    out = nc.dram_tensor("out", [ST, H], F32, kind="ExternalOutput")
    xT, wq, wk, wv, wo, out = (t.ap() for t in (xT, wq, wk, wv, wo, out))

    with tile.TileContext(nc) as tc:
        for _ in range(reps):
            _emit(tc, n_kt, xT, wq, wk, wv, wo, out)
    nc.compile()
    return nc


PHASES = os.environ.get("KPHASES", "pao")  # p=proj a=attn o=outproj


def _emit(tc, n_kt, xT, wq, wk, wv, wo, out):
    nc = tc.nc
    ctx_pools = []

    def pool(name, bufs, space="SBUF"):
        p = tc.alloc_tile_pool(name=name, bufs=bufs, space=space)
        ctx_pools.append(p)
        return p

    # --- pools -----------------------------------------------------------
    xp = pool("x", n_kt * NSB)                 # x tiles [128, 512] bf16
    pw = pool("w", 3 * n_kt)                   # weight tiles [128, 128] bf16
    pwo = pool("wo", 1)                        # [128, 1024] bf16
    pqk = pool("qk", 2)                        # qT/kT [128, 4096] bf16
    pv = pool("v", 1)                          # v [128, 32, 128] bf16
    pctxsb = pool("ctxT", 1)                   # ctxT [128, 4096] bf16
    pones = pool("ones", 1)                    # [128, 64] bf16
    pexp = pool("exp", 6)                      # expT [128, 1024] bf16
    prec = pool("rec", 2)                      # recT [128, 512] f32
    pout = pool("outsb", 4)                    # out staging [128, 1024] f32
    # PSUM: big pool 3x2 banks + attention pool 2x1 bank = 8 banks
    PP = pool("pp", 3, space="PSUM")           # [128, 1024] f32
    PA = pool("pa", 2, space="PSUM")           # [128, 512] f32

    # --- load weights and x ---------------------------------------------
    wo_t = pwo.tile([128, H], BF16)
    nc.sync.dma_start(wo_t[:], wo[:, :])
    wq_t, wk_t, wv_t = [], [], []
    for kt in range(n_kt):
        for lst, src in ((wq_t, wq), (wk_t, wk), (wv_t, wv)):
            t = pw.tile([128, DSH], BF16)
            nc.sync.dma_start(t[:], src[kt * 128:(kt + 1) * 128, :])
            lst.append(t)

    x_t = [[None] * NSB for _ in range(n_kt)]
    for sb in range(NSB):
        for kt in range(n_kt):
            t = xp.tile([128, 512], BF16)
            nc.sync.dma_start(t[:], xT[kt * 128:(kt + 1) * 128,
                                       sb * 512:(sb + 1) * 512])
            x_t[kt][sb] = t

    ones_t = pones.tile([128, 64], BF16)
    nc.vector.memset(ones_t[:], 1.0)

    qT = pqk.tile([128, ST], BF16, tag="qk")
    kT = pqk.tile([128, ST], BF16, tag="qk")
    v_sb = pv.tile([128, NSB * 4, DSH], BF16)
    ctxT = pctxsb.tile([128, ST], BF16)

    # --- projections -----------------------------------------------------
    # qT/kT: out[d(128), s] ; lhsT = w[kt] [h,d], rhs = x[kt][sb] [h,s]
    for sbp in range(NSB // 2 if "p" in PHASES else 0):                # pairs of s-blocks share a tile
        for w_list, dst in ((wq_t, qT), (wk_t, kT)):
            ps = PP.tile([128, 1024], F32, tag="pp")
            for half in range(2):
                sb = sbp * 2 + half
                for kt in range(n_kt):
                    nc.tensor.matmul(ps[:, half * 512:(half + 1) * 512],
                                     w_list[kt][:], x_t[kt][sb][:],
                                     start=(kt == 0), stop=(kt == n_kt - 1))
            nc.any.tensor_copy(dst[:, sbp * 1024:(sbp + 1) * 1024], ps[:])
        # v: out[s(128), d(128)] ; lhsT = x[kt][sb] slice [h, s128], rhs = wv[kt]
        ps = PP.tile([128, 1024], F32, tag="pp")
        for half in range(2):
            sb = sbp * 2 + half
            for ssb in range(4):
                o = half * 512 + ssb * 128
                for kt in range(n_kt):
                    nc.tensor.matmul(ps[:, o:o + 128],
                                     x_t[kt][sb][:, ssb * 128:(ssb + 1) * 128],
                                     wv_t[kt][:],
                                     start=(kt == 0), stop=(kt == n_kt - 1))
        nc.any.tensor_copy(v_sb[:, sbp * 8:(sbp + 1) * 8, :], ps[:])

    # --- attention + out-projection, per (batch, q-block) ---------------
    for b in range(B if ("a" in PHASES or "o" in PHASES) else 0):
        for qb in range(NQB):
            q0 = b * S + qb * 512              # global column of this q-block
            ctx_ps = PA.tile([128, 512], F32, tag="pa")
            den_ps = PA.tile([128, 512], F32, tag="pa")
            if "a" not in PHASES:
                nc.vector.memset(ctx_ps[:], 1.0)
                nc.vector.memset(den_ps[:], 1.0)
            for ktp in range(NKT_S // 2 if "a" in PHASES else 0):
                expt = []
                for h in range(2):             # scores, row-packed pairs
                    sc = PP.tile([128, 1024], F32, tag="pp")
                    for j in range(2):
                        kt = ktp * 2 + j
                        k0 = b * S + kt * 128
                        nc.tensor.matmul(
                            sc[:, j * 512:(j + 1) * 512],
                            kT[h * 64:(h + 1) * 64, k0:k0 + 128],
                            qT[h * 64:(h + 1) * 64, q0:q0 + 512],
                            start=True, stop=True)
                    e = pexp.tile([128, 1024], BF16)
                    nc.scalar.activation(e[:], sc[:],
                                         mybir.ActivationFunctionType.Exp,
                                         scale=0.125)
                    expt.append(e)
                for j in range(2):             # ctx + denom accumulate
                    kt = ktp * 2 + j
                    g = b * NKT_S + kt
                    st = (ktp == 0 and j == 0)
                    sp = (ktp == NKT_S // 2 - 1 and j == 1)
                    for h in range(2):
                        nc.tensor.matmul(ctx_ps[h * 64:(h + 1) * 64, :],
                                         v_sb[:, g, h * 64:(h + 1) * 64],
                                         expt[h][:, j * 512:(j + 1) * 512],
                                         start=st, stop=sp)
                    for h in range(2):
                        nc.tensor.matmul(den_ps[h * 64:(h + 1) * 64, :],
                                         ones_t[:],
                                         expt[h][:, j * 512:(j + 1) * 512],
                                         start=st, stop=sp)
            rec = prec.tile([128, 512], F32)
            nc.vector.reciprocal(rec[:], den_ps[:])
            nc.vector.tensor_mul(ctxT[:, q0:q0 + 512], ctx_ps[:], rec[:])

            # out rows q0..q0+512 : lhsT = ctxT col-slice, rhs = wo
            for ssb in range(4 if "o" in PHASES else 0):
                c0 = q0 + ssb * 128
                ps = PP.tile([128, 1024], F32, tag="pp")
                for e in range(2):
                    nc.tensor.matmul(ps[:, e * 512:(e + 1) * 512],
                                     ctxT[:, c0:c0 + 128],
                                     wo_t[:, e * 512:(e + 1) * 512],
                                     start=True, stop=True)
                ot = pout.tile([128, H], F32)
                nc.any.tensor_copy(ot[:], ps[:])
                nc.sync.dma_start(out[c0:c0 + 128, :], ot[:])

    for p in reversed(ctx_pools):
        p.release()


_CACHE = {}


def _get_nc(n_kt):
    if n_kt not in _CACHE:
        _CACHE[n_kt] = _build(n_kt)
    return _CACHE[n_kt]


def _prep_inputs(hidden_states, Wq, bq, Wk, bk, Wv, bv, Wo, bo):
    x = np.ascontiguousarray(np.asarray(hidden_states, np.float32)
                             .reshape(ST, H))
    bias = not (np.all(bq == 0) and np.all(bk == 0) and np.all(bv == 0))
    n_kt = 9 if bias else 8
    xTn = np.zeros((n_kt * 128, ST), np.float32)
    xTn[:H] = x.T
    if bias:
        xTn[H] = 1.0
    xTn = xTn.astype(NPBF16)

    in_maps = []
    for c in range(NCORES):
        rows = slice(c * DSH, (c + 1) * DSH)
        m = {"xT": xTn}
        for name, W, bvec in (("wq", Wq, bq), ("wk", Wk, bk), ("wv", Wv, bv)):
            wt = np.zeros((n_kt * 128, DSH), np.float32)
            wt[:H] = np.asarray(W, np.float32)[rows, :].T
            if bias:
                wt[H] = np.asarray(bvec, np.float32)[rows]
            m[name] = wt.astype(NPBF16)
        m["wo"] = np.ascontiguousarray(
            np.asarray(Wo, np.float32)[:, rows].T).astype(NPBF16)
        in_maps.append(m)
    return n_kt, in_maps


def kernel(hidden_states, Wq, bq, Wk, bk, Wv, bv, Wo, bo, _return_extras=False):
    n_kt, in_maps = _prep_inputs(hidden_states, Wq, bq, Wk, bk, Wv, bv, Wo, bo)
    nc = _get_nc(n_kt)
    res = bass_utils.run_bass_kernel_spmd(nc, in_maps,
                                          core_ids=list(range(NCORES)))
    acc = res.results[0]["out"].astype(np.float64)
    for c in range(1, NCORES):
        acc += res.results[c]["out"]
    acc += np.asarray(bo, np.float64)
    outv = acc.astype(np.float32).reshape(B, S, H)
    if _return_extras:
        return outv, (nc, in_maps, res)
    return outv
